# revision 15
# baseline (speedup 1.0000x reference)
"""Trainium2 Bass kernel for nn_Decoder_46170898432436 (8 NeuronCores).

Distribution:
  - attention (energy / softmax / context): data-parallel over batch B
    (8 batches per core)
  - LSTM gates: tensor-parallel over the 4H gate dim (each core owns a
    128-wide h-slice of each of i/f/g/o); context exchanged via AllGather
  - vocab projection + log_softmax: tensor-parallel over vocab
    (4000 vocab rows per core); h exchanged via AllGather; global
    softmax denominator via AllGather of per-core sums

Host wrapper only reslices / transposes inputs (no FLOPs). Precision:
bf16 operands with fp32 PSUM accumulation for the large matmuls,
fp32 elementwise everywhere else.

Self-contained: hardcodes all shapes from the problem spec.
"""

import os

import numpy as np

import concourse.bass as bass
import concourse.mybir as mybir
import concourse.tile as tile
from concourse import bacc
from concourse.bass_utils import run_bass_kernel_spmd
from concourse.masks import make_identity

F32 = mybir.dt.float32
BF16 = mybir.dt.bfloat16
I32 = mybir.dt.int32
AX = mybir.AxisListType
OP = mybir.AluOpType
AF = mybir.ActivationFunctionType

S, B, H, E, V = 128, 64, 1024, 512, 32000
NCORES = 8
BC = B // NCORES            # 8 batches per core (attention shard)
HC = H // NCORES            # 128-wide h-slice per core (gate shard)
VC = V // NCORES            # 4000 vocab rows per core
KF = 2 * H + E              # 2560 x-features for the fused gate matmul
KXT = KF // 128 + 1         # 21 k-tiles (+1 const tile carrying gate bias)
VN = 8                      # vocab column chunks per core
VCN = VC // VN              # 500 columns per chunk
VKT = 9                     # vocab k-tiles (8 h-tiles + 1 bias tile)

_CACHE = {}
LAST_RESULTS = None


def _build_program():
    nc = bacc.Bacc("TRN2", target_bir_lowering=False, debug=False,
                   num_devices=NCORES)

    # ---- DRAM I/O (values supplied per core by the host wrapper) ----
    d_encT = nc.dram_tensor("encT", [H, BC * S], BF16, kind="ExternalInput")
    d_encN = nc.dram_tensor("encN", [BC * S, H], BF16, kind="ExternalInput")
    d_w2t = nc.dram_tensor("w2t", [H, H], BF16, kind="ExternalInput")
    d_w1t = nc.dram_tensor("w1t", [H, H], BF16, kind="ExternalInput")
    d_wabr = nc.dram_tensor("wabr", [1, H], BF16, kind="ExternalInput")
    d_ind9 = nc.dram_tensor("ind9", [9, BC * S], BF16, kind="ExternalInput")
    d_vt = nc.dram_tensor("vt", [128, 8], BF16, kind="ExternalInput")
    d_ht = nc.dram_tensor("ht", [H, B], BF16, kind="ExternalInput")
    d_htb = nc.dram_tensor("htb", [H, BC], BF16, kind="ExternalInput")
    d_cprev = nc.dram_tensor("cprev", [B, HC], F32, kind="ExternalInput")
    d_wcat = nc.dram_tensor("wcat", [KXT * 128, 4 * HC], BF16,
                            kind="ExternalInput")
    d_owT = nc.dram_tensor("owT", [VKT * 128, VC], BF16, kind="ExternalInput")
    d_emb = nc.dram_tensor("emb", [V, E], F32, kind="ExternalInput")
    d_widx = nc.dram_tensor("widx", [B, 1], I32, kind="ExternalInput")

    d_logp = nc.dram_tensor("logp", [B, VC], F32, kind="ExternalOutput")
    d_hout = nc.dram_tensor("h_out", [B, HC], F32, kind="ExternalOutput")
    d_cout = nc.dram_tensor("c_out", [B, HC], F32, kind="ExternalOutput")

    # ---- internal DRAM for collectives ----
    d_xp = [nc.dram_tensor(f"xp{i}", [BC, 512], BF16) for i in range(2)]
    d_xa = [nc.dram_tensor(f"xa{i}", [B, 512], BF16, addr_space="Shared")
            for i in range(2)]
    d_hpiece = nc.dram_tensor("hpiece", [HC, B], BF16)
    d_hall = nc.dram_tensor("hall", [H, B], BF16, addr_space="Shared")
    d_mspiece = nc.dram_tensor("mspiece", [B, 1], F32)
    d_msall = nc.dram_tensor("msall", [NCORES, B, 1], F32, addr_space="Shared")

    groups = [list(range(NCORES))]

    with tile.TileContext(nc) as tc:
        with (
            tc.tile_pool(name="consts", bufs=1) as cp,
            tc.tile_pool(name="scratch", bufs=2) as sp,
        ):
            # ======== constants / small loads (HWDGE ring) ========
            identity = cp.tile([128, 128], F32)
            make_identity(nc, identity[:])
            ident_bf = cp.tile([128, 128], BF16)
            nc.vector.tensor_copy(out=ident_bf[:], in_=identity[:])
            ones_col = cp.tile([128, 1], BF16)
            nc.gpsimd.memset(ones_col[:], 1.0)
            cst_bf = cp.tile([128, B], BF16)
            nc.gpsimd.memset(cst_bf[:], 0.0)
            nc.gpsimd.memset(cst_bf[0:1, :], 1.0)

            widx_s = cp.tile([B, 1], I32)
            nc.sync.dma_start(out=widx_s[:], in_=d_widx.ap())
            cprev_s = cp.tile([B, HC], F32)
            nc.sync.dma_start(out=cprev_s[:], in_=d_cprev.ap())

            # ======== embedding gather for ALL batches (early) ========
            emb_nat = cp.tile([B, E], F32)
            nc.gpsimd.indirect_dma_start(
                out=emb_nat[:],
                out_offset=None,
                in_=d_emb.ap(),
                in_offset=bass.IndirectOffsetOnAxis(ap=widx_s[:, :1], axis=0),
            )

            # ======== big loads ========
            # SWDGE ring (gpsimd): fp32 -> bf16 cast on the fly, in
            # consumption order: htb/vt/ht/ind9, (encT_k, w2t_k, w1t_k)*8,
            # wcat, owT.
            htb_s = cp.tile([128, 8, BC], BF16)
            nc.sync.dma_start(out=htb_s[:],
                              in_=d_htb.ap().rearrange("(t p) b -> p t b", p=128))
            vt_s = cp.tile([128, 8], BF16)
            nc.sync.dma_start(out=vt_s[:], in_=d_vt.ap())
            ht_s = cp.tile([128, 8, B], BF16)
            nc.sync.dma_start(out=ht_s[:],
                              in_=d_ht.ap().rearrange("(t p) b -> p t b", p=128))
            ind9_s = cp.tile([9, BC * S], BF16)
            nc.sync.dma_start(out=ind9_s[:], in_=d_ind9.ap())
            # row 8 of hpw (the W_a bias row) straight from DRAM
            hpw = cp.tile([9, H], BF16)
            nc.sync.dma_start(out=hpw[8:9, :], in_=d_wabr.ap())

            encT_s = cp.tile([128, 8, BC * S], BF16)
            w2t_s = cp.tile([128, 8, H], BF16)
            w1k_tiles = []
            for k in range(8):
                nc.sync.dma_start(out=encT_s[:, k, :],
                                  in_=d_encT.ap()[k * 128:(k + 1) * 128, :])
                nc.sync.dma_start(out=w2t_s[:, k, :],
                                  in_=d_w2t.ap()[k * 128:(k + 1) * 128, :])
                w1k = sp.tile([128, H], BF16, tag="w1k", bufs=2,
                              name=f"w1k{k}")
                nc.sync.dma_start(out=w1k[:],
                                  in_=d_w1t.ap()[k * 128:(k + 1) * 128, :])
                w1k_tiles.append(w1k)
            # natural-layout encoder copy for the context matmul
            encN_s = cp.tile([128, 8, H], BF16)
            for k in range(8):
                nc.sync.dma_start(out=encN_s[:, k, :],
                                  in_=d_encN.ap()[k * 128:(k + 1) * 128, :])
            wcat_s = cp.tile([128, KXT, 4 * HC], BF16)
            nc.sync.dma_start(out=wcat_s[:],
                              in_=d_wcat.ap().rearrange("(t p) g -> p t g", p=128))
            owT_s = cp.tile([128, VKT, VC], BF16)
            for k in range(VKT):
                nc.sync.dma_start(out=owT_s[:, k, :],
                                  in_=d_owT.ap()[k * 128:(k + 1) * 128, :])

            # ======== phase 1: h_part = hidden_b @ W1^T ========
            with tc.tile_pool(name="ps1", bufs=1, space="PSUM") as pp1:
                ps_hp0 = pp1.tile([BC, 512], F32, tag="hp0", name="ps_hp0")
                ps_hp1 = pp1.tile([BC, 512], F32, tag="hp1", name="ps_hp1")
                for k in range(8):
                    nc.tensor.matmul(out=ps_hp0[:], lhsT=htb_s[:, k, :],
                                     rhs=w1k_tiles[k][:, 0:512],
                                     start=(k == 0), stop=(k == 7))
                    nc.tensor.matmul(out=ps_hp1[:], lhsT=htb_s[:, k, :],
                                     rhs=w1k_tiles[k][:, 512:1024],
                                     start=(k == 0), stop=(k == 7))
                nc.vector.tensor_copy(out=hpw[0:BC, 0:512], in_=ps_hp0[:])
                nc.vector.tensor_copy(out=hpw[0:BC, 512:1024], in_=ps_hp1[:])

                # embedded^T tiles for the gates matmul (local, all-B)
                xt_s = cp.tile([128, 12, B], BF16)
                for t in range(4):
                    ps_e = pp1.tile([128, B], F32, tag="tr", bufs=2,
                                    name=f"ps_e{t}")
                    nc.tensor.transpose(
                        out=ps_e[:], in_=emb_nat[:, t * 128:(t + 1) * 128],
                        identity=identity[:B, :B])
                    nc.scalar.activation(out=xt_s[:, 8 + t, :], in_=ps_e[:],
                                         func=AF.Copy)

            # ======== phase 2: energy + h_part-fold + tanh + v-dot + pad ====
            lgs = cp.tile([1, BC * S], F32)
            pds = cp.tile([1, BC * S], F32)
            with tc.tile_pool(name="ps2", bufs=1, space="PSUM") as pp2:
                ps_lg = [pp2.tile([1, 512], F32, tag=f"lg{n}", name=f"ps_lg{n}")
                         for n in range(2)]
                for m in range(8):
                    for n in range(2):
                        ns = slice(n * 512, (n + 1) * 512)
                        pe = pp2.tile([128, 512], F32, tag="pe", bufs=3,
                                      name=f"pe{m}_{n}")
                        for k in range(8):
                            nc.tensor.matmul(
                                out=pe[:],
                                lhsT=w2t_s[:, k, m * 128:(m + 1) * 128],
                                rhs=encT_s[:, k, ns],
                                start=(k == 0), stop=False)
                        # + (h_part + W_a bias) broadcast over s, via the
                        # indicator matrix as a 9-row extra contraction tile
                        nc.tensor.matmul(
                            out=pe[:],
                            lhsT=hpw[:, m * 128:(m + 1) * 128],
                            rhs=ind9_s[:, ns],
                            start=False, stop=True)
                        etan = sp.tile([128, 512], BF16, tag="etan", bufs=2,
                                       name=f"etan{m}_{n}")
                        nc.scalar.activation(out=etan[:], in_=pe[:],
                                             func=AF.Tanh)
                        nc.tensor.matmul(out=ps_lg[n][:],
                                         lhsT=vt_s[:, m:m + 1], rhs=etan[:],
                                         start=(m == 0), stop=(m == 7))
                # pad-row sums over h (ones-dot); encT fully resident by now
                ps_pd = [pp2.tile([1, 512], F32, tag=f"pd{n}", name=f"ps_pd{n}")
                         for n in range(2)]
                for n in range(2):
                    for k in range(8):
                        nc.tensor.matmul(
                            out=ps_pd[n][:], lhsT=ones_col[:],
                            rhs=encT_s[:, k, n * 512:(n + 1) * 512],
                            start=(k == 0), stop=(k == 7))
                nc.vector.tensor_copy(out=lgs[:, 0:512], in_=ps_lg[0][:])
                nc.vector.tensor_copy(out=lgs[:, 512:1024], in_=ps_lg[1][:])
                nc.vector.tensor_copy(out=pds[:, 0:512], in_=ps_pd[0][:])
                nc.vector.tensor_copy(out=pds[:, 512:1024], in_=ps_pd[1][:])

            # ======== softmax over s (per local batch) ========
            t1k = cp.tile([1, BC * S], F32)
            t2k = cp.tile([1, BC * S], F32)
            # mask = (rowsum == 0); logits += -1e5 * mask
            nc.vector.tensor_scalar(out=t1k[:], in0=pds[:], scalar1=0.0,
                                    scalar2=None, op0=OP.is_equal)
            nc.vector.scalar_tensor_tensor(out=t2k[:], in0=t1k[:],
                                           scalar=-100000.0, in1=lgs[:],
                                           op0=OP.mult, op1=OP.add)
            lg3 = t2k[:].rearrange("p (b s) -> p b s", s=S)
            mx = cp.tile([1, BC], F32)
            nc.vector.tensor_reduce(out=mx[:], in_=lg3, axis=AX.X, op=OP.max)
            nc.vector.tensor_tensor(
                out=t1k[:].rearrange("p (b s) -> p b s", s=S),
                in0=lg3,
                in1=mx[:].unsqueeze(-1).broadcast_to([1, BC, S]),
                op=OP.subtract)
            nc.scalar.activation(out=t2k[:], in_=t1k[:], func=AF.Exp)
            sm = cp.tile([1, BC], F32)
            nc.vector.tensor_reduce(out=sm[:],
                                    in_=t2k[:].rearrange("p (b s) -> p b s", s=S),
                                    axis=AX.X, op=OP.add)
            rec = cp.tile([1, BC], F32)
            nc.vector.reciprocal(out=rec[:], in_=sm[:])
            attn = cp.tile([1, BC * S], F32)
            nc.vector.tensor_tensor(
                out=attn[:].rearrange("p (b s) -> p b s", s=S),
                in0=t2k[:].rearrange("p (b s) -> p b s", s=S),
                in1=rec[:].unsqueeze(-1).broadcast_to([1, BC, S]),
                op=OP.mult)

            with tc.tile_pool(name="ps3", bufs=1, space="PSUM") as pp3:
                # block-diagonal attention matrix [s, b] per batch tile
                abl = cp.tile([128, 8, 8], BF16)
                nc.gpsimd.memset(abl[:], 0.0)
                for b in range(BC):
                    ps_a = pp3.tile([128, 1], F32, tag="at", bufs=1,
                                    name=f"ps_a{b}")
                    nc.tensor.transpose(out=ps_a[:],
                                        in_=attn[:, b * S:(b + 1) * S],
                                        identity=identity[:1, :1])
                    nc.scalar.activation(out=abl[:, b, b:b + 1], in_=ps_a[:],
                                         func=AF.Copy)

                # ==== context as PE matmul over natural-layout enc ====
                # out[b, h] = sum_s attn[s,b] * enc[s,b,h], k-tiles = batches
                for hc2 in range(2):
                    hs = slice(hc2 * 512, (hc2 + 1) * 512)
                    ps_ctx = pp3.tile([BC, 512], F32, tag="ctx", bufs=2,
                                      name=f"ps_ctx{hc2}")
                    for k in range(8):
                        nc.tensor.matmul(out=ps_ctx[:], lhsT=abl[:, k, :],
                                         rhs=encN_s[:, k, hs],
                                         start=(k == 0), stop=(k == 7))
                    piece = sp.tile([BC, 512], BF16, tag="piece", bufs=2,
                                    name=f"piece{hc2}")
                    nc.scalar.activation(out=piece[:], in_=ps_ctx[:],
                                         func=AF.Copy)
                    # exchange this half right away (first AG absorbs skew)
                    nc.sync.dma_start(out=d_xp[hc2].ap(), in_=piece[:])
                    nc.gpsimd.collective_compute(
                        "AllGather", OP.bypass, replica_groups=groups,
                        ins=[d_xp[hc2].ap()], outs=[d_xa[hc2].ap()])

                # ==== gates matmul (TP over gate rows) ====
                # order: const bias tile, h_prev tiles, emb tiles first (all
                # local - they run during the AllGather), context tiles last.
                ps_g = pp3.tile([B, 4 * HC], F32, tag="g", name="ps_g")
                korder = [20] + list(range(12, 20)) + list(range(8, 12)) \
                    + list(range(0, 8))
                # context^T tiles from the gathered pieces (bf16 transposes)
                for half in range(2):
                    xall_s = sp.tile([B, 512], BF16, tag="xall", bufs=2,
                                     name=f"xall{half}")
                    nc.sync.dma_start(out=xall_s[:], in_=d_xa[half].ap())
                    for t in range(4):
                        ps_x = pp3.tile([128, B], BF16, tag="trb", bufs=2,
                                        name=f"ps_x{half}_{t}")
                        nc.tensor.transpose(
                            out=ps_x[:],
                            in_=xall_s[:, t * 128:(t + 1) * 128],
                            identity=ident_bf[:B, :B])
                        nc.vector.tensor_copy(out=xt_s[:, half * 4 + t, :],
                                              in_=ps_x[:])
                for i, k in enumerate(korder):
                    if k < 12:
                        lhsT = xt_s[:, k, :]
                    elif k < 20:
                        lhsT = ht_s[:, k - 12, :]
                    else:
                        lhsT = cst_bf[:]
                    nc.tensor.matmul(out=ps_g[:], lhsT=lhsT,
                                     rhs=wcat_s[:, k, :],
                                     start=(i == 0), stop=(i == KXT - 1))

                # ==== LSTM cell (own h-slice, all 64 batches) ====
                i_s = cp.tile([B, HC], F32)
                f_s = cp.tile([B, HC], F32)
                g_t = cp.tile([B, HC], F32)
                o_s = cp.tile([B, HC], F32)
                nc.scalar.activation(out=i_s[:], in_=ps_g[:, 0:128],
                                     func=AF.Sigmoid)
                nc.scalar.activation(out=f_s[:], in_=ps_g[:, 128:256],
                                     func=AF.Sigmoid)
                nc.scalar.activation(out=g_t[:], in_=ps_g[:, 256:384],
                                     func=AF.Tanh)
                nc.scalar.activation(out=o_s[:], in_=ps_g[:, 384:512],
                                     func=AF.Sigmoid)
                t1 = cp.tile([B, HC], F32)
                nc.vector.tensor_tensor(out=t1[:], in0=f_s[:], in1=cprev_s[:],
                                        op=OP.mult)
                t2 = cp.tile([B, HC], F32)
                nc.vector.tensor_tensor(out=t2[:], in0=i_s[:], in1=g_t[:],
                                        op=OP.mult)
                c_new = cp.tile([B, HC], F32)
                nc.vector.tensor_tensor(out=c_new[:], in0=t1[:], in1=t2[:],
                                        op=OP.add)
                tanh_c = cp.tile([B, HC], F32)
                nc.scalar.activation(out=tanh_c[:], in_=c_new[:], func=AF.Tanh)
                h_new = cp.tile([B, HC], F32)
                nc.vector.tensor_tensor(out=h_new[:], in0=o_s[:],
                                        in1=tanh_c[:], op=OP.mult)
                nc.sync.dma_start(out=d_cout.ap(), in_=c_new[:])
                nc.sync.dma_start(out=d_hout.ap(), in_=h_new[:])

                # ==== exchange h pieces ====
                ps_h = pp3.tile([HC, B], F32, tag="tr", bufs=2, name="ps_h")
                nc.tensor.transpose(out=ps_h[:], in_=h_new[:],
                                    identity=identity[:B, :B])
                hpc = cp.tile([HC, B], BF16)
                nc.scalar.activation(out=hpc[:], in_=ps_h[:], func=AF.Copy)
                nc.sync.dma_start(out=d_hpiece.ap(), in_=hpc[:])
                nc.gpsimd.collective_compute(
                    "AllGather", OP.bypass, replica_groups=groups,
                    ins=[d_hpiece.ap()], outs=[d_hall.ap()])
                htn_s = cp.tile([128, 8, B], BF16)
                nc.sync.dma_start(
                    out=htn_s[:],
                    in_=d_hall.ap().rearrange("(t p) b -> p t b", p=128))

            # ======== phase 4: vocab projection + log_softmax ========
            logits = cp.tile([B, VC], F32)
            with tc.tile_pool(name="ps4", bufs=1, space="PSUM") as pp4:
                ps_v = [pp4.tile([B, VCN], F32, tag=f"v{n}", name=f"ps_v{n}")
                        for n in range(VN)]
                # bias tile (k=8) first: it has no dependency on the h
                # exchange and runs during the AllGather.
                for i, k in enumerate([8] + list(range(8))):
                    lhsT = htn_s[:, k, :] if k < 8 else cst_bf[:]
                    for n in range(VN):
                        nc.tensor.matmul(
                            out=ps_v[n][:], lhsT=lhsT,
                            rhs=owT_s[:, k, n * VCN:(n + 1) * VCN],
                            start=(i == 0), stop=(i == VKT - 1))

                # local sum-exp (logits are tiny: no max shift needed)
                ssp = cp.tile([B, VN], F32)
                for n in range(VN):
                    scr = sp.tile([B, VCN], F32, tag="scr", bufs=2,
                                  name=f"scr{n}")
                    nc.scalar.activation(out=scr[:], in_=ps_v[n][:],
                                         func=AF.Exp,
                                         accum_out=ssp[:, n:n + 1])
                s1 = cp.tile([B, 1], F32)
                nc.vector.tensor_reduce(out=s1[:], in_=ssp[:], axis=AX.X,
                                        op=OP.add)

                # exchange per-core sums; combine into global log-denominator
                nc.sync.dma_start(out=d_mspiece.ap(), in_=s1[:])
                nc.gpsimd.collective_compute(
                    "AllGather", OP.bypass, replica_groups=groups,
                    ins=[d_mspiece.ap()], outs=[d_msall.ap()])
                msl = cp.tile([B, NCORES], F32)
                nc.sync.dma_start(
                    out=msl[:],
                    in_=d_msall.ap().rearrange("c b v -> b (c v)"))
                gs = cp.tile([B, 1], F32)
                nc.vector.tensor_reduce(out=gs[:], in_=msl[:], axis=AX.X,
                                        op=OP.add)
                negk = cp.tile([B, 1], F32)
                nc.scalar.activation(out=negk[:], in_=gs[:], func=AF.Ln)
                nc.vector.tensor_scalar(out=negk[:], in0=negk[:], scalar1=-1.0,
                                        scalar2=None, op0=OP.mult)
                # logp = logits - log(sum) straight out of PSUM, store+DMA
                for n in range(VN):
                    ns = slice(n * VCN, (n + 1) * VCN)
                    nc.vector.tensor_scalar(out=logits[:, ns], in0=ps_v[n][:],
                                            scalar1=negk[:, :1], scalar2=None,
                                            op0=OP.add)
                    nc.sync.dma_start(out=d_logp.ap()[:, ns],
                                      in_=logits[:, ns])

    nc.compile()
    return nc


def _prep_inputs(hidden_h, hidden_c, encoder_outputs, last_word, embedding,
                 W_a_w, W_a_b, v_w, v_b, W_ih, W_hh, b_ih, b_hh, out_w, out_b):
    import ml_dtypes
    bf = ml_dtypes.bfloat16
    f = np.float32
    enc = np.asarray(encoder_outputs, f)
    hh = np.asarray(hidden_h, f)[0]           # [B, H]
    hc = np.asarray(hidden_c, f)[0]           # [B, H]
    Waw = np.asarray(W_a_w, f)                # [H, 2H]
    Wab = np.asarray(W_a_b, f)                # [H]
    vw = np.asarray(v_w, f)                   # [1, H]
    Wih = np.asarray(W_ih, f)                 # [4H, E+H]
    Whh = np.asarray(W_hh, f)                 # [4H, H]
    bsum = np.asarray(b_ih, f) + np.asarray(b_hh, f)  # [4H]
    ow = np.asarray(out_w, f)                 # [V, H]
    obf = np.asarray(out_b, f)                # [V]
    lw = np.asarray(last_word).reshape(-1)    # [B]
    emb = np.ascontiguousarray(np.asarray(embedding, f))

    w2t = np.ascontiguousarray(Waw[:, H:].T.astype(bf))
    w1t = np.ascontiguousarray(Waw[:, :H].T.astype(bf))
    wabr = np.ascontiguousarray(Wab.reshape(1, H).astype(bf))
    vt_t = np.ascontiguousarray(vw[0].reshape(8, 128).T.astype(bf))
    ht = np.ascontiguousarray(hh.T.astype(bf))
    Wcat = np.concatenate([Wih, Whh], axis=1)  # [4H, 2560]
    ind9 = np.zeros((9, BC * S), bf)
    for b in range(BC):
        ind9[b, b * S:(b + 1) * S] = 1.0
    ind9[8, :] = 1.0
    widx = np.ascontiguousarray(lw.astype(np.int32).reshape(B, 1))

    in_maps = []
    for c in range(NCORES):
        bsl = slice(c * BC, (c + 1) * BC)
        encT_c = np.ascontiguousarray(
            enc[:, bsl, :].transpose(2, 1, 0).reshape(H, BC * S).astype(bf))
        encN_c = np.ascontiguousarray(
            enc[:, bsl, :].transpose(1, 0, 2).reshape(BC * S, H).astype(bf))
        rows = np.concatenate(
            [np.arange(g * H + c * HC, g * H + (c + 1) * HC) for g in range(4)])
        wcat_c = np.zeros((KXT * 128, 4 * HC), bf)
        wcat_c[:KF] = Wcat[rows].T
        wcat_c[KF] = bsum[rows]
        owT_c = np.zeros((VKT * 128, VC), bf)
        owT_c[:H] = ow[c * VC:(c + 1) * VC].T
        owT_c[H] = obf[c * VC:(c + 1) * VC]
        in_maps.append({
            "encT": encT_c,
            "encN": encN_c,
            "w2t": w2t,
            "w1t": w1t,
            "wabr": wabr,
            "ind9": ind9,
            "vt": vt_t,
            "ht": ht,
            "htb": np.ascontiguousarray(hh[bsl].T.astype(bf)),
            "cprev": np.ascontiguousarray(hc[:, c * HC:(c + 1) * HC]),
            "wcat": wcat_c,
            "owT": owT_c,
            "emb": emb,
            "widx": widx,
        })
    return in_maps


def kernel(**inputs):
    global LAST_RESULTS
    if "nc" not in _CACHE:
        _CACHE["nc"] = _build_program()
    nc = _CACHE["nc"]
    in_maps = _prep_inputs(**inputs)
    trace = bool(int(os.environ.get("DECODER_TRACE", "0")))
    res = run_bass_kernel_spmd(nc, in_maps, list(range(NCORES)), trace=trace)
    LAST_RESULTS = res
    logp = np.concatenate([res.results[c]["logp"] for c in range(NCORES)],
                          axis=1)[None]
    h = np.concatenate([res.results[c]["h_out"] for c in range(NCORES)],
                       axis=1)[None]
    c = np.concatenate([res.results[c]["c_out"] for c in range(NCORES)],
                       axis=1)[None]
    return logp.astype(np.float32), h.astype(np.float32), c.astype(np.float32)


# revision 16
# speedup vs baseline: 1.1478x; 1.1478x over previous
"""Trainium2 Bass kernel for nn_Decoder_46170898432436 (8 NeuronCores).

Distribution:
  - attention (energy / softmax / context): data-parallel over batch B
    (8 batches per core)
  - LSTM gates: tensor-parallel over the 4H gate dim (each core owns a
    128-wide h-slice of each of i/f/g/o); context exchanged via AllGather
  - vocab projection + log_softmax: tensor-parallel over vocab
    (4000 vocab rows per core); h exchanged via AllGather; global
    softmax denominator via AllGather of per-core sums

Host wrapper only reslices / transposes inputs (no FLOPs). Precision:
bf16 operands with fp32 PSUM accumulation for the large matmuls,
fp32 elementwise everywhere else.

Self-contained: hardcodes all shapes from the problem spec.
"""

import os

import numpy as np

import concourse.bass as bass
import concourse.mybir as mybir
import concourse.tile as tile
from concourse import bacc
from concourse.bass_utils import run_bass_kernel_spmd
from concourse.masks import make_identity

F32 = mybir.dt.float32
BF16 = mybir.dt.bfloat16
I32 = mybir.dt.int32
AX = mybir.AxisListType
OP = mybir.AluOpType
AF = mybir.ActivationFunctionType

S, B, H, E, V = 128, 64, 1024, 512, 32000
NCORES = 8
BC = B // NCORES            # 8 batches per core (attention shard)
HC = H // NCORES            # 128-wide h-slice per core (gate shard)
VC = V // NCORES            # 4000 vocab rows per core
KF = 2 * H + E              # 2560 x-features for the fused gate matmul
KXT = KF // 128 + 1         # 21 k-tiles (+1 const tile carrying gate bias)
VN = 8                      # vocab column chunks per core
VCN = VC // VN              # 500 columns per chunk
VKT = 9                     # vocab k-tiles (8 h-tiles + 1 bias tile)

_CACHE = {}
LAST_RESULTS = None


def _build_program():
    nc = bacc.Bacc("TRN2", target_bir_lowering=False, debug=False,
                   num_devices=NCORES)

    # ---- DRAM I/O (values supplied per core by the host wrapper) ----
    d_encT = nc.dram_tensor("encT", [H, BC * S], BF16, kind="ExternalInput")
    d_encN = nc.dram_tensor("encN", [BC * S, H], BF16, kind="ExternalInput")
    d_w2t = nc.dram_tensor("w2t", [H, H], BF16, kind="ExternalInput")
    d_w1t = nc.dram_tensor("w1t", [H, H], BF16, kind="ExternalInput")
    d_wabr = nc.dram_tensor("wabr", [1, H], BF16, kind="ExternalInput")
    d_ind9 = nc.dram_tensor("ind9", [9, BC * S], BF16, kind="ExternalInput")
    d_vt = nc.dram_tensor("vt", [128, 8], BF16, kind="ExternalInput")
    d_ht = nc.dram_tensor("ht", [H, B], BF16, kind="ExternalInput")
    d_htb = nc.dram_tensor("htb", [H, BC], BF16, kind="ExternalInput")
    d_cprev = nc.dram_tensor("cprev", [B, HC], F32, kind="ExternalInput")
    d_wcat = nc.dram_tensor("wcat", [KXT * 128, 4 * HC], BF16,
                            kind="ExternalInput")
    d_owT = nc.dram_tensor("owT", [VKT * 128, VC], BF16, kind="ExternalInput")
    d_emb = nc.dram_tensor("emb", [V, E], F32, kind="ExternalInput")
    d_widx = nc.dram_tensor("widx", [B, 1], I32, kind="ExternalInput")

    d_logp = nc.dram_tensor("logp", [B, VC], F32, kind="ExternalOutput")
    d_hout = nc.dram_tensor("h_out", [B, HC], F32, kind="ExternalOutput")
    d_cout = nc.dram_tensor("c_out", [B, HC], F32, kind="ExternalOutput")

    # ---- internal DRAM for collectives ----
    d_xp = [nc.dram_tensor(f"xp{i}", [BC, 512], BF16) for i in range(2)]
    d_xa = [nc.dram_tensor(f"xa{i}", [B, 512], BF16, addr_space="Shared")
            for i in range(2)]
    d_hpiece = nc.dram_tensor("hpiece", [HC, B], BF16)
    d_hall = nc.dram_tensor("hall", [H, B], BF16, addr_space="Shared")
    d_mspiece = nc.dram_tensor("mspiece", [B, 1], F32)
    d_msall = nc.dram_tensor("msall", [NCORES, B, 1], F32, addr_space="Shared")

    groups = [list(range(NCORES))]

    with tile.TileContext(nc) as tc:
        with (
            tc.tile_pool(name="consts", bufs=1) as cp,
            tc.tile_pool(name="scratch", bufs=2) as sp,
        ):
            # ======== constants / small loads (HWDGE ring) ========
            identity = cp.tile([128, 128], F32)
            make_identity(nc, identity[:])
            ident_bf = cp.tile([128, 128], BF16)
            nc.vector.tensor_copy(out=ident_bf[:], in_=identity[:])
            ones_col = cp.tile([128, 1], BF16)
            nc.gpsimd.memset(ones_col[:], 1.0)
            cst_bf = cp.tile([128, B], BF16)
            nc.gpsimd.memset(cst_bf[:], 0.0)
            nc.gpsimd.memset(cst_bf[0:1, :], 1.0)

            widx_s = cp.tile([B, 1], I32)
            nc.sync.dma_start(out=widx_s[:], in_=d_widx.ap())
            cprev_s = cp.tile([B, HC], F32)
            nc.sync.dma_start(out=cprev_s[:], in_=d_cprev.ap())

            # ======== embedding gather for ALL batches (early) ========
            emb_nat = cp.tile([B, E], F32)
            nc.gpsimd.indirect_dma_start(
                out=emb_nat[:],
                out_offset=None,
                in_=d_emb.ap(),
                in_offset=bass.IndirectOffsetOnAxis(ap=widx_s[:, :1], axis=0),
            )

            # ======== big loads ========
            # SWDGE ring (gpsimd): fp32 -> bf16 cast on the fly, in
            # consumption order: htb/vt/ht/ind9, (encT_k, w2t_k, w1t_k)*8,
            # wcat, owT.
            htb_s = cp.tile([128, 8, BC], BF16)
            nc.sync.dma_start(out=htb_s[:],
                              in_=d_htb.ap().rearrange("(t p) b -> p t b", p=128))
            vt_s = cp.tile([128, 8], BF16)
            nc.sync.dma_start(out=vt_s[:], in_=d_vt.ap())
            ht_s = cp.tile([128, 8, B], BF16)
            nc.sync.dma_start(out=ht_s[:],
                              in_=d_ht.ap().rearrange("(t p) b -> p t b", p=128))
            ind9_s = cp.tile([9, BC * S], BF16)
            nc.sync.dma_start(out=ind9_s[:], in_=d_ind9.ap())
            # row 8 of hpw (the W_a bias row) straight from DRAM
            hpw = cp.tile([9, H], BF16)
            nc.sync.dma_start(out=hpw[8:9, :], in_=d_wabr.ap())

            encT_s = cp.tile([128, 8, BC * S], BF16)
            w2t_s = cp.tile([128, 8, H], BF16)
            w1k_tiles = []
            for k in range(8):
                nc.sync.dma_start(out=encT_s[:, k, :],
                                  in_=d_encT.ap()[k * 128:(k + 1) * 128, :])
                nc.sync.dma_start(out=w2t_s[:, k, :],
                                  in_=d_w2t.ap()[k * 128:(k + 1) * 128, :])
                w1k = sp.tile([128, H], BF16, tag="w1k", bufs=2,
                              name=f"w1k{k}")
                nc.sync.dma_start(out=w1k[:],
                                  in_=d_w1t.ap()[k * 128:(k + 1) * 128, :])
                w1k_tiles.append(w1k)
            # natural-layout encoder copy for the context matmul
            encN_s = cp.tile([128, 8, H], BF16)
            for k in range(8):
                nc.sync.dma_start(out=encN_s[:, k, :],
                                  in_=d_encN.ap()[k * 128:(k + 1) * 128, :])
            wcat_s = cp.tile([128, KXT, 4 * HC], BF16)
            nc.sync.dma_start(out=wcat_s[:],
                              in_=d_wcat.ap().rearrange("(t p) g -> p t g", p=128))
            owT_s = cp.tile([128, VKT, VC], BF16)
            for k in range(VKT):
                nc.sync.dma_start(out=owT_s[:, k, :],
                                  in_=d_owT.ap()[k * 128:(k + 1) * 128, :])

            # ======== phase 1: h_part = hidden_b @ W1^T ========
            with tc.tile_pool(name="ps1", bufs=1, space="PSUM") as pp1:
                ps_hp0 = pp1.tile([BC, 512], F32, tag="hp0", name="ps_hp0")
                ps_hp1 = pp1.tile([BC, 512], F32, tag="hp1", name="ps_hp1")
                for k in range(8):
                    nc.tensor.matmul(out=ps_hp0[:], lhsT=htb_s[:, k, :],
                                     rhs=w1k_tiles[k][:, 0:512],
                                     start=(k == 0), stop=(k == 7))
                    nc.tensor.matmul(out=ps_hp1[:], lhsT=htb_s[:, k, :],
                                     rhs=w1k_tiles[k][:, 512:1024],
                                     start=(k == 0), stop=(k == 7))
                nc.vector.tensor_copy(out=hpw[0:BC, 0:512], in_=ps_hp0[:])
                nc.vector.tensor_copy(out=hpw[0:BC, 512:1024], in_=ps_hp1[:])

                # embedded^T tiles for the gates matmul (local, all-B)
                xt_s = cp.tile([128, 12, B], BF16)
                for t in range(4):
                    ps_e = pp1.tile([128, B], F32, tag="tr", bufs=2,
                                    name=f"ps_e{t}")
                    nc.tensor.transpose(
                        out=ps_e[:], in_=emb_nat[:, t * 128:(t + 1) * 128],
                        identity=identity[:B, :B])
                    nc.scalar.activation(out=xt_s[:, 8 + t, :], in_=ps_e[:],
                                         func=AF.Copy)

            # ======== phase 2: energy + h_part-fold + tanh + v-dot + pad ====
            lgs = cp.tile([1, BC * S], F32)
            pds = cp.tile([1, BC * S], F32)
            with tc.tile_pool(name="ps2", bufs=1, space="PSUM") as pp2:
                ps_lg = [pp2.tile([1, 512], F32, tag=f"lg{n}", name=f"ps_lg{n}")
                         for n in range(2)]
                for m in range(8):
                    for n in range(2):
                        ns = slice(n * 512, (n + 1) * 512)
                        pe = pp2.tile([128, 512], F32, tag="pe", bufs=3,
                                      name=f"pe{m}_{n}")
                        for k in range(8):
                            nc.tensor.matmul(
                                out=pe[:],
                                lhsT=w2t_s[:, k, m * 128:(m + 1) * 128],
                                rhs=encT_s[:, k, ns],
                                start=(k == 0), stop=False)
                        # + (h_part + W_a bias) broadcast over s, via the
                        # indicator matrix as a 9-row extra contraction tile
                        nc.tensor.matmul(
                            out=pe[:],
                            lhsT=hpw[:, m * 128:(m + 1) * 128],
                            rhs=ind9_s[:, ns],
                            start=False, stop=True)
                        etan = sp.tile([128, 512], BF16, tag="etan", bufs=2,
                                       name=f"etan{m}_{n}")
                        nc.scalar.activation(out=etan[:], in_=pe[:],
                                             func=AF.Tanh)
                        nc.tensor.matmul(out=ps_lg[n][:],
                                         lhsT=vt_s[:, m:m + 1], rhs=etan[:],
                                         start=(m == 0), stop=(m == 7))
                # pad-row sums over h (ones-dot); encT fully resident by now
                ps_pd = [pp2.tile([1, 512], F32, tag=f"pd{n}", name=f"ps_pd{n}")
                         for n in range(2)]
                for n in range(2):
                    for k in range(8):
                        nc.tensor.matmul(
                            out=ps_pd[n][:], lhsT=ones_col[:],
                            rhs=encT_s[:, k, n * 512:(n + 1) * 512],
                            start=(k == 0), stop=(k == 7))
                nc.vector.tensor_copy(out=lgs[:, 0:512], in_=ps_lg[0][:])
                nc.vector.tensor_copy(out=lgs[:, 512:1024], in_=ps_lg[1][:])
                nc.vector.tensor_copy(out=pds[:, 0:512], in_=ps_pd[0][:])
                nc.vector.tensor_copy(out=pds[:, 512:1024], in_=ps_pd[1][:])

            # ======== softmax over s (per local batch) ========
            t1k = cp.tile([1, BC * S], F32)
            t2k = cp.tile([1, BC * S], F32)
            # mask = (rowsum == 0); logits += -1e5 * mask
            nc.vector.tensor_scalar(out=t1k[:], in0=pds[:], scalar1=0.0,
                                    scalar2=None, op0=OP.is_equal)
            nc.vector.scalar_tensor_tensor(out=t2k[:], in0=t1k[:],
                                           scalar=-100000.0, in1=lgs[:],
                                           op0=OP.mult, op1=OP.add)
            lg3 = t2k[:].rearrange("p (b s) -> p b s", s=S)
            mx = cp.tile([1, BC], F32)
            nc.vector.tensor_reduce(out=mx[:], in_=lg3, axis=AX.X, op=OP.max)
            nc.vector.tensor_tensor(
                out=t1k[:].rearrange("p (b s) -> p b s", s=S),
                in0=lg3,
                in1=mx[:].unsqueeze(-1).broadcast_to([1, BC, S]),
                op=OP.subtract)
            nc.scalar.activation(out=t2k[:], in_=t1k[:], func=AF.Exp)
            sm = cp.tile([1, BC], F32)
            nc.vector.tensor_reduce(out=sm[:],
                                    in_=t2k[:].rearrange("p (b s) -> p b s", s=S),
                                    axis=AX.X, op=OP.add)
            rec = cp.tile([1, BC], F32)
            nc.vector.reciprocal(out=rec[:], in_=sm[:])
            attn = cp.tile([1, BC * S], F32)
            nc.vector.tensor_tensor(
                out=attn[:].rearrange("p (b s) -> p b s", s=S),
                in0=t2k[:].rearrange("p (b s) -> p b s", s=S),
                in1=rec[:].unsqueeze(-1).broadcast_to([1, BC, S]),
                op=OP.mult)

            with tc.tile_pool(name="ps3", bufs=1, space="PSUM") as pp3:
                # block-diagonal attention matrix [s, b] per batch tile
                abl = cp.tile([128, 8, 8], BF16)
                nc.gpsimd.memset(abl[:], 0.0)
                for b in range(BC):
                    ps_a = pp3.tile([128, 1], F32, tag="at", bufs=1,
                                    name=f"ps_a{b}")
                    nc.tensor.transpose(out=ps_a[:],
                                        in_=attn[:, b * S:(b + 1) * S],
                                        identity=identity[:1, :1])
                    nc.scalar.activation(out=abl[:, b, b:b + 1], in_=ps_a[:],
                                         func=AF.Copy)

                # ==== context as PE matmul over natural-layout enc ====
                # out[b, h] = sum_s attn[s,b] * enc[s,b,h], k-tiles = batches
                for hc2 in range(2):
                    hs = slice(hc2 * 512, (hc2 + 1) * 512)
                    ps_ctx = pp3.tile([BC, 512], F32, tag="ctx", bufs=2,
                                      name=f"ps_ctx{hc2}")
                    for k in range(8):
                        nc.tensor.matmul(out=ps_ctx[:], lhsT=abl[:, k, :],
                                         rhs=encN_s[:, k, hs],
                                         start=(k == 0), stop=(k == 7))
                    piece = sp.tile([BC, 512], BF16, tag="piece", bufs=2,
                                    name=f"piece{hc2}")
                    nc.scalar.activation(out=piece[:], in_=ps_ctx[:],
                                         func=AF.Copy)
                    # exchange this half right away (first AG absorbs skew)
                    nc.sync.dma_start(out=d_xp[hc2].ap(), in_=piece[:])
                    nc.gpsimd.collective_compute(
                        "AllGather", OP.bypass, replica_groups=groups,
                        ins=[d_xp[hc2].ap()], outs=[d_xa[hc2].ap()])

                # ==== gates matmul (TP over gate rows) ====
                # order: const bias tile, h_prev tiles, emb tiles first (all
                # local - they run during the AllGather), context tiles last.
                ps_g = pp3.tile([B, 4 * HC], F32, tag="g", name="ps_g")
                korder = [20] + list(range(12, 20)) + list(range(8, 12)) \
                    + list(range(0, 8))
                # context^T tiles from the gathered pieces (bf16 transposes)
                for half in range(2):
                    xall_s = sp.tile([B, 512], BF16, tag="xall", bufs=2,
                                     name=f"xall{half}")
                    nc.sync.dma_start(out=xall_s[:], in_=d_xa[half].ap())
                    for t in range(4):
                        ps_x = pp3.tile([128, B], BF16, tag="trb", bufs=2,
                                        name=f"ps_x{half}_{t}")
                        nc.tensor.transpose(
                            out=ps_x[:],
                            in_=xall_s[:, t * 128:(t + 1) * 128],
                            identity=ident_bf[:B, :B])
                        nc.vector.tensor_copy(out=xt_s[:, half * 4 + t, :],
                                              in_=ps_x[:])
                for i, k in enumerate(korder):
                    if k < 12:
                        lhsT = xt_s[:, k, :]
                    elif k < 20:
                        lhsT = ht_s[:, k - 12, :]
                    else:
                        lhsT = cst_bf[:]
                    nc.tensor.matmul(out=ps_g[:], lhsT=lhsT,
                                     rhs=wcat_s[:, k, :],
                                     start=(i == 0), stop=(i == KXT - 1))

                # ==== LSTM cell (own h-slice, all 64 batches) ====
                i_s = cp.tile([B, HC], F32)
                f_s = cp.tile([B, HC], F32)
                g_t = cp.tile([B, HC], F32)
                o_s = cp.tile([B, HC], F32)
                nc.scalar.activation(out=i_s[:], in_=ps_g[:, 0:128],
                                     func=AF.Sigmoid)
                nc.scalar.activation(out=f_s[:], in_=ps_g[:, 128:256],
                                     func=AF.Sigmoid)
                nc.scalar.activation(out=g_t[:], in_=ps_g[:, 256:384],
                                     func=AF.Tanh)
                nc.scalar.activation(out=o_s[:], in_=ps_g[:, 384:512],
                                     func=AF.Sigmoid)
                t1 = cp.tile([B, HC], F32)
                nc.vector.tensor_tensor(out=t1[:], in0=f_s[:], in1=cprev_s[:],
                                        op=OP.mult)
                t2 = cp.tile([B, HC], F32)
                nc.vector.tensor_tensor(out=t2[:], in0=i_s[:], in1=g_t[:],
                                        op=OP.mult)
                c_new = cp.tile([B, HC], F32)
                nc.vector.tensor_tensor(out=c_new[:], in0=t1[:], in1=t2[:],
                                        op=OP.add)
                tanh_c = cp.tile([B, HC], F32)
                nc.scalar.activation(out=tanh_c[:], in_=c_new[:], func=AF.Tanh)
                h_new = cp.tile([B, HC], F32)
                nc.vector.tensor_tensor(out=h_new[:], in0=o_s[:],
                                        in1=tanh_c[:], op=OP.mult)
                nc.sync.dma_start(out=d_cout.ap(), in_=c_new[:])
                nc.sync.dma_start(out=d_hout.ap(), in_=h_new[:])

                # ==== exchange h pieces ====
                ps_h = pp3.tile([HC, B], F32, tag="tr", bufs=2, name="ps_h")
                nc.tensor.transpose(out=ps_h[:], in_=h_new[:],
                                    identity=identity[:B, :B])
                hpc = cp.tile([HC, B], BF16)
                nc.scalar.activation(out=hpc[:], in_=ps_h[:], func=AF.Copy)
                nc.sync.dma_start(out=d_hpiece.ap(), in_=hpc[:])
                nc.gpsimd.collective_compute(
                    "AllGather", OP.bypass, replica_groups=groups,
                    ins=[d_hpiece.ap()], outs=[d_hall.ap()])
                htn_s = cp.tile([128, 8, B], BF16)
                nc.sync.dma_start(
                    out=htn_s[:],
                    in_=d_hall.ap().rearrange("(t p) b -> p t b", p=128))

            # ======== phase 4: vocab projection + log_softmax ========
            logits = cp.tile([B, VC], F32)
            with tc.tile_pool(name="ps4", bufs=1, space="PSUM") as pp4:
                ps_v = [pp4.tile([B, VCN], F32, tag=f"v{n}", name=f"ps_v{n}")
                        for n in range(VN)]
                # bias tile (k=8) first: it has no dependency on the h
                # exchange and runs during the AllGather.
                for i, k in enumerate([8] + list(range(8))):
                    lhsT = htn_s[:, k, :] if k < 8 else cst_bf[:]
                    for n in range(VN):
                        nc.tensor.matmul(
                            out=ps_v[n][:], lhsT=lhsT,
                            rhs=owT_s[:, k, n * VCN:(n + 1) * VCN],
                            start=(i == 0), stop=(i == VKT - 1))

                # local sum-exp (logits are tiny: no max shift needed)
                ssp = cp.tile([B, VN], F32)
                for n in range(VN):
                    scr = sp.tile([B, VCN], F32, tag="scr", bufs=2,
                                  name=f"scr{n}")
                    nc.scalar.activation(out=scr[:], in_=ps_v[n][:],
                                         func=AF.Exp,
                                         accum_out=ssp[:, n:n + 1])
                s1 = cp.tile([B, 1], F32)
                nc.vector.tensor_reduce(out=s1[:], in_=ssp[:], axis=AX.X,
                                        op=OP.add)

                # exchange per-core sums; combine into global log-denominator
                nc.sync.dma_start(out=d_mspiece.ap(), in_=s1[:])
                nc.gpsimd.collective_compute(
                    "AllGather", OP.bypass, replica_groups=groups,
                    ins=[d_mspiece.ap()], outs=[d_msall.ap()])
                msl = cp.tile([B, NCORES], F32)
                nc.sync.dma_start(
                    out=msl[:],
                    in_=d_msall.ap().rearrange("c b v -> b (c v)"))
                gs = cp.tile([B, 1], F32)
                nc.vector.tensor_reduce(out=gs[:], in_=msl[:], axis=AX.X,
                                        op=OP.add)
                negk = cp.tile([B, 1], F32)
                nc.scalar.activation(out=negk[:], in_=gs[:], func=AF.Ln)
                nc.vector.tensor_scalar(out=negk[:], in0=negk[:], scalar1=-1.0,
                                        scalar2=None, op0=OP.mult)
                # logp = logits - log(sum) straight out of PSUM, store+DMA
                for n in range(VN):
                    ns = slice(n * VCN, (n + 1) * VCN)
                    nc.vector.tensor_scalar(out=logits[:, ns], in0=ps_v[n][:],
                                            scalar1=negk[:, :1], scalar2=None,
                                            op0=OP.add)
                    nc.sync.dma_start(out=d_logp.ap()[:, ns],
                                      in_=logits[:, ns])

    nc.compile()
    return nc


def _prep_inputs(hidden_h, hidden_c, encoder_outputs, last_word, embedding,
                 W_a_w, W_a_b, v_w, v_b, W_ih, W_hh, b_ih, b_hh, out_w, out_b):
    import ml_dtypes
    bf = ml_dtypes.bfloat16
    f = np.float32
    enc = np.asarray(encoder_outputs, f)
    hh = np.asarray(hidden_h, f)[0]           # [B, H]
    hc = np.asarray(hidden_c, f)[0]           # [B, H]
    Waw = np.asarray(W_a_w, f)                # [H, 2H]
    Wab = np.asarray(W_a_b, f)                # [H]
    vw = np.asarray(v_w, f)                   # [1, H]
    Wih = np.asarray(W_ih, f)                 # [4H, E+H]
    Whh = np.asarray(W_hh, f)                 # [4H, H]
    bsum = np.asarray(b_ih, f) + np.asarray(b_hh, f)  # [4H]
    ow = np.asarray(out_w, f)                 # [V, H]
    obf = np.asarray(out_b, f)                # [V]
    lw = np.asarray(last_word).reshape(-1)    # [B]
    emb = np.ascontiguousarray(np.asarray(embedding, f))

    w2t = np.ascontiguousarray(Waw[:, H:].T.astype(bf))
    w1t = np.ascontiguousarray(Waw[:, :H].T.astype(bf))
    wabr = np.ascontiguousarray(Wab.reshape(1, H).astype(bf))
    vt_t = np.ascontiguousarray(vw[0].reshape(8, 128).T.astype(bf))
    ht = np.ascontiguousarray(hh.T.astype(bf))
    Wcat = np.concatenate([Wih, Whh], axis=1)  # [4H, 2560]
    ind9 = np.zeros((9, BC * S), bf)
    for b in range(BC):
        ind9[b, b * S:(b + 1) * S] = 1.0
    ind9[8, :] = 1.0
    widx = np.ascontiguousarray(lw.astype(np.int32).reshape(B, 1))

    in_maps = []
    for c in range(NCORES):
        bsl = slice(c * BC, (c + 1) * BC)
        encT_c = np.ascontiguousarray(
            enc[:, bsl, :].transpose(2, 1, 0).reshape(H, BC * S).astype(bf))
        encN_c = np.ascontiguousarray(
            enc[:, bsl, :].transpose(1, 0, 2).reshape(BC * S, H).astype(bf))
        rows = np.concatenate(
            [np.arange(g * H + c * HC, g * H + (c + 1) * HC) for g in range(4)])
        wcat_c = np.zeros((KXT * 128, 4 * HC), bf)
        wcat_c[:KF] = Wcat[rows].T
        wcat_c[KF] = bsum[rows]
        owT_c = np.zeros((VKT * 128, VC), bf)
        owT_c[:H] = ow[c * VC:(c + 1) * VC].T
        owT_c[H] = obf[c * VC:(c + 1) * VC]
        in_maps.append({
            "encT": encT_c,
            "encN": encN_c,
            "w2t": w2t,
            "w1t": w1t,
            "wabr": wabr,
            "ind9": ind9,
            "vt": vt_t,
            "ht": ht,
            "htb": np.ascontiguousarray(hh[bsl].T.astype(bf)),
            "cprev": np.ascontiguousarray(hc[:, c * HC:(c + 1) * HC]),
            "wcat": wcat_c,
            "owT": owT_c,
            "emb": emb,
            "widx": widx,
        })
    return in_maps


def kernel(**inputs):
    global LAST_RESULTS
    if "nc" not in _CACHE:
        _CACHE["nc"] = _build_program()
    nc = _CACHE["nc"]
    in_maps = _prep_inputs(**inputs)
    trace = bool(int(os.environ.get("DECODER_TRACE", "0")))
    kw = {}
    if int(os.environ.get("DECODER_TRACE_ALL", "0")):
        kw["trace_cores"] = list(range(NCORES))
    res = run_bass_kernel_spmd(nc, in_maps, list(range(NCORES)), trace=trace,
                               **kw)
    LAST_RESULTS = res
    logp = np.concatenate([res.results[c]["logp"] for c in range(NCORES)],
                          axis=1)[None]
    h = np.concatenate([res.results[c]["h_out"] for c in range(NCORES)],
                       axis=1)[None]
    c = np.concatenate([res.results[c]["c_out"] for c in range(NCORES)],
                       axis=1)[None]
    return logp.astype(np.float32), h.astype(np.float32), c.astype(np.float32)
